# revision 20
# baseline (speedup 1.0000x reference)
"""Trainium2 Bass kernel for nn_LinearCondensed.

Computes out[b, o] = sum_k weight[o, k] * x[b, indx_seqs[o, k]] + bias[o]
with B=2048, IN_F=OUT_F=4096, FAN_IN=32.

Strategy: the gather has no fast on-chip primitive (any materialized gather
moves 32x the data of x itself), so we densify the sparse weight matrix on
the host -- W'[o, i] = sum_{k: indx_seqs[o,k]==i} weight[o, k] -- and run a
dense matmul out = x @ W'^T + bias on the PE array. OUT_F is sharded 8 ways
across cores (512 columns each), x is replicated.

v3:
  * bf16 operands: ~24MB HBM traffic per core (PE-bound), FWL hides LDWEIGHTS.
  * Dummy warm-up matmuls on scratch SBUF run while the first DMAs land, so
    the HAM clock-gate is already at 2.4GHz when real matmuls start.
  * First x tile and the first 8 weight k-tiles ride in 0.25MB pieces,
    byte-interleaved ~1:1, so the PE's first real matmul fires ~3.5us after
    DMA start and the wavefront is rarely starved during the 4MB W stream.
  * Phase 1 wavefront over b-tiles 0-2 with 3 live PSUM banks follows the
    arrival schedule; phase 2 is a pure k-inner stream.
"""

import os
import sys
import types

import ml_dtypes
import numpy as np

import concourse.bacc as bacc
import concourse.mybir as mybir
import concourse.tile as tile
from concourse.bass_utils import run_bass_kernel_spmd

B, IN_F, OUT_F, FAN_IN = 2048, 4096, 4096, 32
NCORES = 8
OSH = OUT_F // NCORES          # 512 output features per core
P = 128                        # partitions
BT = B // P                    # 16 batch tiles
KT = IN_F // P                 # 32 contraction tiles
N = OSH                        # 512 moving columns
WG = 4                         # k-tiles per weight DMA group
NG = KT // WG                  # 8 weight groups
NWARM = 13                     # scratch matmuls to warm the PE clock gate

f32 = mybir.dt.float32
bf16 = mybir.dt.bfloat16

_cache = {}


def _enable_ntff_hook():
    """Register the ctypes NTFF profile hook (the image's antenv lacks
    axon_hooks); lets trace=True produce a neuron-profile under axon."""
    try:
        from antenv.axon_hooks import get_axon_ntff_profile_hook  # noqa: F401
        return
    except ImportError:
        pass
    try:
        import antenv
        from trn_agent_boot.trn_boot import _ntff_profile_via_ctypes

        mod = types.ModuleType("antenv.axon_hooks")
        holder = [None]
        mod.set_axon_ntff_profile_hook = lambda h: holder.__setitem__(0, h)
        mod.get_axon_ntff_profile_hook = lambda: holder[0]
        antenv.axon_hooks = mod
        sys.modules["antenv.axon_hooks"] = mod
        mod.set_axon_ntff_profile_hook(
            _ntff_profile_via_ctypes("/opt/axon/libaxon_pjrt.so"))
        import concourse.bass_utils as bu
        bu.upload_artifacts = lambda tmpdir: str(tmpdir)
    except Exception:
        pass


def _build():
    nc = bacc.Bacc()
    # Layouts (host-pretiled, all contiguous):
    #   XT[t, p, a, c]    = x[t*128 + c, a*128 + p]    -> per b-tile t: [128, KT*128]
    #   WT[g, p, j, n]    = W'[o0 + n, (g*4+j)*128+p]  -> per group g: [128, 4*512]
    #   BIAS[p, n]        = bias[o0 + n]               (replicated across p)
    XT = nc.declare_dram_parameter("XT", [BT, P, KT * P], bf16, isOutput=False)
    WT = nc.declare_dram_parameter("WT", [NG, P, WG * N], bf16, isOutput=False)
    BIAS = nc.declare_dram_parameter("BIAS", [P, N], f32, isOutput=False)
    OUT = nc.declare_dram_parameter("OUT", [B, N], f32, isOutput=True)

    XTv = XT.ap().rearrange("t p (a c) -> t p a c", a=KT)
    WTv = WT.ap().rearrange("g p (j n) -> g p j n", j=WG)

    with tile.TileContext(nc) as tc:
        with (
            tc.tile_pool(name="wpool", bufs=1) as wpool,
            tc.tile_pool(name="xpool", bufs=1) as xpool,
            tc.tile_pool(name="cpool", bufs=1) as cpool,
            tc.tile_pool(name="opool", bufs=4) as opool,
            tc.tile_pool(name="psum", bufs=5, space="PSUM") as psum,
        ):
            # Everything statically allocated (16MB x + 4MB W bf16 fits SBUF)
            # so no ring-reuse dependency can ever stall a DMA.
            xtiles = {}
            # weight SBUF pieces keyed by ktile -> (tile, sub-index)
            wmap = [None] * KT

            # --- Two HWDGE queues: W quads on sync, x tiles on scalar.
            # The SDMA engines round-robin between the two rings at packet
            # granularity, so W and x interleave ~1:1 by bytes and every
            # engine has twice the outstanding descriptors (hides HBM
            # latency). x0 rides in a 0.25MB quarter + 0.75MB remainder so
            # the first matmul fires as early as possible.
            xq0 = xpool.tile([P, 8, P], bf16, name="xq0", tag="xq0")
            xq123 = xpool.tile([P, 24, P], bf16, name="xq123", tag="xq123")

            def load_wquad(g):  # ktiles 4g..4g+3
                w = wpool.tile([P, WG, N], bf16, name=f"wq{g}", tag=f"wq{g}")
                nc.sync.dma_start(w[:], WTv[g])
                for j in range(WG):
                    wmap[4 * g + j] = (w, j)

            def load_x(t, ring=False):
                if ring:
                    # 3-slot ring: the DMA trigger waits until the slot's
                    # previous occupant was fully consumed by the PE, so the
                    # x stream trickles at consumption rate (~150GB/s)
                    # instead of stealing half the HBM bandwidth from W.
                    xs = xpool.tile([P, KT, P], bf16, name=f"xs{t}", tag="xs",
                                    bufs=2)
                else:
                    xs = xpool.tile([P, KT, P], bf16, name=f"xs{t}",
                                    tag=f"xs{t}")
                nc.scalar.dma_start(xs[:], XTv[t])
                xtiles[t] = xs

            nc.scalar.dma_start(xq0[:], XTv[0][:, 0:8, :])
            nc.scalar.dma_start(xq123[:], XTv[0][:, 8:KT, :])
            for t in range(1, 4):
                load_x(t)
            for t in range(4, BT):
                load_x(t, ring=True)
            for g in range(NG):
                load_wquad(g)
            brow = cpool.tile([P, N], f32)
            nc.sync.dma_start(brow[:], BIAS[:])

            def xsl(t, a):
                if t == 0:
                    return xq0[:, a, :] if a < 8 else xq123[:, a - 8, :]
                return xtiles[t][:, a, :]

            accs = {}

            def mm(t, a0, a1):
                for a in range(a0, a1):
                    w, j = wmap[a]
                    nc.tensor.matmul(
                        accs[t][:], xsl(t, a), w[:, j, :],
                        start=(a == 0), stop=(a == KT - 1),
                    )

            def finish(t):
                osb = opool.tile([P, N], f32, name=f"osb{t}", tag="osb")
                nc.vector.tensor_tensor(osb[:], accs[t][:], brow[:],
                                        mybir.AluOpType.add)
                # Stores ride the SWDGE (gpsimd) queue: no contention with
                # the W stream (sync) or the x stream (scalar). The last two
                # tiles use the scalar HWDGE queue (idle by then, ~1.5us
                # lower completion latency) to shorten the kernel tail.
                if t >= BT - 2:
                    nc.scalar.dma_start(OUT.ap()[t * P:(t + 1) * P, :], osb[:])
                else:
                    nc.gpsimd.dma_start(OUT.ap()[t * P:(t + 1) * P, :], osb[:])

            # --- PE warm-up: junk matmuls on scratch SBUF while the first
            # DMAs are in flight, so HAM reaches 2.4GHz before real work.
            scr = cpool.tile([P, N], bf16, name="scr")
            nc.vector.memset(scr[:], 0.0)
            pscr = psum.tile([P, N], f32, name="pscr", tag="pscr", bufs=1)
            for _ in range(NWARM):
                nc.tensor.matmul(pscr[:], scr[:, :P], scr[:],
                                 start=True, stop=True)

            # --- Phase 1 wavefront: b-tiles join one per weight quad (with
            # k-catch-up), so during the W stream the PE always has work from
            # every tile already resident. 6 live PSUM banks + scratch = 7.
            WAVE = 4
            join = {0: 0, 1: 1, 2: 1, 3: 2}  # tile -> joining quad
            for t in range(WAVE):
                accs[t] = psum.tile([P, N], f32, name=f"acc{t}", tag="acc",
                                    bufs=WAVE)
            done = {t: 0 for t in range(WAVE)}

            def run_to(t, a1):
                if done[t] < a1:
                    mm(t, done[t], a1)
                    done[t] = a1

            for g in range(NG):
                for t in range(WAVE):
                    if join[t] <= g:
                        run_to(t, 4 * (g + 1))
            for t in range(WAVE):
                finish(t)

            # --- Phase 2: remaining b-tiles, k-inner, x already streaming ---
            for t in range(WAVE, BT):
                accs[t] = psum.tile([P, N], f32, name=f"accp{t}", tag="acc",
                                    bufs=WAVE)
                mm(t, 0, KT)
                finish(t)

    nc.compile()
    return nc


def kernel(x, weight, bias, indx_seqs):
    x = np.asarray(x, dtype=np.float32)
    weight = np.asarray(weight, dtype=np.float32)
    bias = np.asarray(bias, dtype=np.float32)
    indx_seqs = np.asarray(indx_seqs)

    if "nc" not in _cache:
        _cache["nc"] = _build()
    nc = _cache["nc"]

    # Densify sparse weights: W'[o, i] += weight[o, k] at i = indx_seqs[o, k]
    wd = np.zeros((OUT_F, IN_F), dtype=np.float32)
    np.add.at(wd, (np.arange(OUT_F)[:, None], indx_seqs), weight)

    # Host pre-tiling into SBUF-friendly layouts (bf16).
    # XT[t, p, a, c] = x[t*128+c, a*128+p]
    xt = np.ascontiguousarray(
        x.reshape(BT, P, KT, P).transpose(0, 3, 2, 1)
    ).reshape(BT, P, KT * P).astype(ml_dtypes.bfloat16)
    in_maps = []
    for c in range(NCORES):
        wshard = wd[c * OSH:(c + 1) * OSH]            # (512, 4096)
        # WT[g, p, j, n] = W'[o0+n, (g*4+j)*128+p]
        wt = np.ascontiguousarray(
            wshard.reshape(OSH, NG, WG, P).transpose(1, 3, 2, 0)
        ).reshape(NG, P, WG * N).astype(ml_dtypes.bfloat16)
        in_maps.append({
            "XT": xt,
            "WT": wt,
            "BIAS": np.ascontiguousarray(
                np.broadcast_to(bias[c * OSH:(c + 1) * OSH], (P, N))),
        })

    trace = bool(int(os.environ.get("BASSK_TRACE", "0"))) or bool(
        os.environ.get("BASS_TRACE"))
    if trace:
        _enable_ntff_hook()
    res = run_bass_kernel_spmd(
        nc, in_maps, list(range(NCORES)), trace=trace,
        trace_cores=list(range(NCORES)) if trace else None,
    )
    _cache["last_results"] = res

    out = np.concatenate([res.results[c]["OUT"] for c in range(NCORES)], axis=1)
    return out


# revision 21
# speedup vs baseline: 1.1668x; 1.1668x over previous
"""Trainium2 Bass kernel for nn_LinearCondensed.

Computes out[b, o] = sum_k weight[o, k] * x[b, indx_seqs[o, k]] + bias[o]
with B=2048, IN_F=OUT_F=4096, FAN_IN=32.

Strategy: the gather has no fast on-chip primitive (any materialized gather
moves 32x the data of x itself), so we densify the sparse weight matrix on
the host -- W'[o, i] = sum_{k: indx_seqs[o,k]==i} weight[o, k] -- and run a
dense matmul out = x @ W'^T + bias on the PE array. OUT_F is sharded 8 ways
across cores (512 columns each), x is replicated.

v3:
  * bf16 operands: ~24MB HBM traffic per core (PE-bound), FWL hides LDWEIGHTS.
  * Dummy warm-up matmuls on scratch SBUF run while the first DMAs land, so
    the HAM clock-gate is already at 2.4GHz when real matmuls start.
  * First x tile and the first 8 weight k-tiles ride in 0.25MB pieces,
    byte-interleaved ~1:1, so the PE's first real matmul fires ~3.5us after
    DMA start and the wavefront is rarely starved during the 4MB W stream.
  * Phase 1 wavefront over b-tiles 0-2 with 3 live PSUM banks follows the
    arrival schedule; phase 2 is a pure k-inner stream.
"""

import os
import sys
import types

import ml_dtypes
import numpy as np

import concourse.bacc as bacc
import concourse.mybir as mybir
import concourse.tile as tile
from concourse.bass_utils import run_bass_kernel_spmd

B, IN_F, OUT_F, FAN_IN = 2048, 4096, 4096, 32
NCORES = 8
OSH = OUT_F // NCORES          # 512 output features per core
P = 128                        # partitions
BT = B // P                    # 16 batch tiles
KT = IN_F // P                 # 32 contraction tiles
N = OSH                        # 512 moving columns
WG = 4                         # k-tiles per weight DMA group
NG = KT // WG                  # 8 weight groups
NWARM = 16                     # scratch matmuls to warm the PE clock gate

f32 = mybir.dt.float32
bf16 = mybir.dt.bfloat16

_cache = {}


def _enable_ntff_hook():
    """Register the ctypes NTFF profile hook (the image's antenv lacks
    axon_hooks); lets trace=True produce a neuron-profile under axon."""
    try:
        from antenv.axon_hooks import get_axon_ntff_profile_hook  # noqa: F401
        return
    except ImportError:
        pass
    try:
        import antenv
        from trn_agent_boot.trn_boot import _ntff_profile_via_ctypes

        mod = types.ModuleType("antenv.axon_hooks")
        holder = [None]
        mod.set_axon_ntff_profile_hook = lambda h: holder.__setitem__(0, h)
        mod.get_axon_ntff_profile_hook = lambda: holder[0]
        antenv.axon_hooks = mod
        sys.modules["antenv.axon_hooks"] = mod
        mod.set_axon_ntff_profile_hook(
            _ntff_profile_via_ctypes("/opt/axon/libaxon_pjrt.so"))
        import concourse.bass_utils as bu
        bu.upload_artifacts = lambda tmpdir: str(tmpdir)
    except Exception:
        pass


def _build():
    nc = bacc.Bacc()
    # Layouts (host-pretiled, all contiguous):
    #   XT[t, p, a, c]    = x[t*128 + c, a*128 + p]    -> per b-tile t: [128, KT*128]
    #   WT[g, p, j, n]    = W'[o0 + n, (g*4+j)*128+p]  -> per group g: [128, 4*512]
    #   BIAS[p, n]        = bias[o0 + n]               (replicated across p)
    XT = nc.declare_dram_parameter("XT", [BT, P, KT * P], bf16, isOutput=False)
    WT = nc.declare_dram_parameter("WT", [NG, P, WG * N], bf16, isOutput=False)
    BIAS = nc.declare_dram_parameter("BIAS", [P, N], f32, isOutput=False)
    OUT = nc.declare_dram_parameter("OUT", [B, N], f32, isOutput=True)

    XTv = XT.ap().rearrange("t p (a c) -> t p a c", a=KT)
    WTv = WT.ap().rearrange("g p (j n) -> g p j n", j=WG)

    with tile.TileContext(nc) as tc:
        with (
            tc.tile_pool(name="wpool", bufs=1) as wpool,
            tc.tile_pool(name="xpool", bufs=1) as xpool,
            tc.tile_pool(name="cpool", bufs=1) as cpool,
            tc.tile_pool(name="opool", bufs=4) as opool,
            tc.tile_pool(name="psum", bufs=5, space="PSUM") as psum,
        ):
            # Everything statically allocated (16MB x + 4MB W bf16 fits SBUF)
            # so no ring-reuse dependency can ever stall a DMA.
            xtiles = {}
            # weight SBUF pieces keyed by ktile -> (tile, sub-index)
            wmap = [None] * KT

            # --- Two HWDGE queues: W quads on sync, x tiles on scalar.
            # The SDMA engines round-robin between the two rings at packet
            # granularity, so W and x interleave ~1:1 by bytes and every
            # engine has twice the outstanding descriptors (hides HBM
            # latency). x0 rides in a 0.25MB quarter + 0.75MB remainder so
            # the first matmul fires as early as possible.
            xq0 = xpool.tile([P, 8, P], bf16, name="xq0", tag="xq0")
            xq123 = xpool.tile([P, 24, P], bf16, name="xq123", tag="xq123")

            def load_wquad(g):  # ktiles 4g..4g+3
                w = wpool.tile([P, WG, N], bf16, name=f"wq{g}", tag=f"wq{g}")
                nc.sync.dma_start(w[:], WTv[g])
                for j in range(WG):
                    wmap[4 * g + j] = (w, j)

            def load_x(t, ring=False):
                if ring:
                    # 3-slot ring: the DMA trigger waits until the slot's
                    # previous occupant was fully consumed by the PE, so the
                    # x stream trickles at consumption rate (~150GB/s)
                    # instead of stealing half the HBM bandwidth from W.
                    xs = xpool.tile([P, KT, P], bf16, name=f"xs{t}", tag="xs",
                                    bufs=3)
                else:
                    xs = xpool.tile([P, KT, P], bf16, name=f"xs{t}",
                                    tag=f"xs{t}")
                nc.scalar.dma_start(xs[:], XTv[t])
                xtiles[t] = xs

            nc.scalar.dma_start(xq0[:], XTv[0][:, 0:8, :])
            nc.scalar.dma_start(xq123[:], XTv[0][:, 8:KT, :])
            for t in range(1, 4):
                load_x(t)
            for t in range(4, BT):
                load_x(t, ring=True)
            for g in range(NG):
                load_wquad(g)
            brow = cpool.tile([P, N], f32)
            nc.sync.dma_start(brow[:], BIAS[:])

            def xsl(t, a):
                if t == 0:
                    return xq0[:, a, :] if a < 8 else xq123[:, a - 8, :]
                return xtiles[t][:, a, :]

            accs = {}

            def mm(t, a0, a1):
                for a in range(a0, a1):
                    w, j = wmap[a]
                    nc.tensor.matmul(
                        accs[t][:], xsl(t, a), w[:, j, :],
                        start=(a == 0), stop=(a == KT - 1),
                    )

            def finish(t):
                osb = opool.tile([P, N], f32, name=f"osb{t}", tag="osb")
                nc.vector.tensor_tensor(osb[:], accs[t][:], brow[:],
                                        mybir.AluOpType.add)
                # Stores ride the SWDGE (gpsimd) queue: no contention with
                # the W stream (sync) or the x stream (scalar). The last two
                # tiles use the scalar HWDGE queue (idle by then, ~1.5us
                # lower completion latency) to shorten the kernel tail.
                nc.gpsimd.dma_start(OUT.ap()[t * P:(t + 1) * P, :], osb[:])

            # --- PE warm-up: junk matmuls on scratch SBUF while the first
            # DMAs are in flight, so HAM reaches 2.4GHz before real work.
            scr = cpool.tile([P, N], bf16, name="scr")
            nc.vector.memset(scr[:], 0.0)
            pscr = psum.tile([P, N], f32, name="pscr", tag="pscr", bufs=1)
            for _ in range(NWARM):
                nc.tensor.matmul(pscr[:], scr[:, :P], scr[:],
                                 start=True, stop=True)

            # --- Phase 1 wavefront: b-tiles join one per weight quad (with
            # k-catch-up), so during the W stream the PE always has work from
            # every tile already resident. 6 live PSUM banks + scratch = 7.
            WAVE = 4
            join = {0: 0, 1: 1, 2: 1, 3: 2}  # tile -> joining quad
            for t in range(WAVE):
                accs[t] = psum.tile([P, N], f32, name=f"acc{t}", tag="acc",
                                    bufs=WAVE)
            done = {t: 0 for t in range(WAVE)}

            def run_to(t, a1):
                if done[t] < a1:
                    mm(t, done[t], a1)
                    done[t] = a1

            for g in range(NG):
                for t in range(WAVE):
                    if join[t] <= g:
                        run_to(t, 4 * (g + 1))
            for t in range(WAVE):
                finish(t)

            # --- Phase 2: remaining b-tiles, k-inner, x already streaming ---
            for t in range(WAVE, BT):
                accs[t] = psum.tile([P, N], f32, name=f"accp{t}", tag="acc",
                                    bufs=WAVE)
                mm(t, 0, KT)
                finish(t)

    nc.compile()
    return nc


def kernel(x, weight, bias, indx_seqs):
    x = np.asarray(x, dtype=np.float32)
    weight = np.asarray(weight, dtype=np.float32)
    bias = np.asarray(bias, dtype=np.float32)
    indx_seqs = np.asarray(indx_seqs)

    if "nc" not in _cache:
        _cache["nc"] = _build()
    nc = _cache["nc"]

    # Densify sparse weights: W'[o, i] += weight[o, k] at i = indx_seqs[o, k]
    wd = np.zeros((OUT_F, IN_F), dtype=np.float32)
    np.add.at(wd, (np.arange(OUT_F)[:, None], indx_seqs), weight)

    # Host pre-tiling into SBUF-friendly layouts (bf16).
    # XT[t, p, a, c] = x[t*128+c, a*128+p]
    xt = np.ascontiguousarray(
        x.reshape(BT, P, KT, P).transpose(0, 3, 2, 1)
    ).reshape(BT, P, KT * P).astype(ml_dtypes.bfloat16)
    in_maps = []
    for c in range(NCORES):
        wshard = wd[c * OSH:(c + 1) * OSH]            # (512, 4096)
        # WT[g, p, j, n] = W'[o0+n, (g*4+j)*128+p]
        wt = np.ascontiguousarray(
            wshard.reshape(OSH, NG, WG, P).transpose(1, 3, 2, 0)
        ).reshape(NG, P, WG * N).astype(ml_dtypes.bfloat16)
        in_maps.append({
            "XT": xt,
            "WT": wt,
            "BIAS": np.ascontiguousarray(
                np.broadcast_to(bias[c * OSH:(c + 1) * OSH], (P, N))),
        })

    trace = bool(int(os.environ.get("BASSK_TRACE", "0"))) or bool(
        os.environ.get("BASS_TRACE"))
    if trace:
        _enable_ntff_hook()
    res = run_bass_kernel_spmd(
        nc, in_maps, list(range(NCORES)), trace=trace,
        trace_cores=list(range(NCORES)) if trace else None,
    )
    _cache["last_results"] = res

    out = np.concatenate([res.results[c]["OUT"] for c in range(NCORES)], axis=1)
    return out
